# revision 1
# baseline (speedup 1.0000x reference)
"""Trainium2 Bass kernel for nn_KAN_DiffPhys_ODE (SIR ODE scan driven by a
tiny RBF-KAN beta(t) schedule).

Strategy (per spec sharding hint): data-parallel over the batch axis B
across 8 NeuronCores. Each core owns B/8 = 4096 trajectories laid out as
[128 partitions, 32 free] (batch index b_loc = f*128 + p, chosen so a
TensorE transpose of a 4-step pack yields DRAM-contiguous 512B rows).

Per scan step t (the serial part, on VectorE only):
    I' = (S * db_t + c) * I      -- one AFFINE_MUL_REDUCE custom DVE op
    S' = (I * -db_t + 1) * S     -- one AFFINE_MUL_REDUCE custom DVE op
with db_t = dt*beta_t broadcast per-partition, c = 1 - dt*gamma.
clip(0,5) is a provable no-op here: I,S stay in [0,1] and every factor is
nonnegative in f32, so it is elided.

Output path (off the critical VE chain): 4 steps pack into [128,128] ->
TensorE transpose -> PSUM -> ScalarE copy into a [128,2048] staging tile
(64 steps) -> 4 HWDGE DMAs with 512B-contiguous descriptors.

beta(t) ([1024] values) is tiny and replicated: computed on host in f64
from the KAN weights (per the sharding hint) and shipped as a [128,1024]
broadcast input so each step's scalar is a [P,1] AP slice.
"""

import numpy as np

import concourse.bacc as bacc
import concourse.bass as bass  # noqa: F401  (AP helpers)
import concourse.mybir as mybir
import concourse.tile as tile
from concourse.bass_utils import run_bass_kernel_spmd

T = 1024
B = 32768
NCORES = 8
BL = B // NCORES          # 4096 per core
P = 128
F = BL // P               # 32 free elems per partition
K_PACK = 4                # steps per [128,128] transpose pack
J_STAGE = 16              # packs per staging tile
STEPS_PER_STAGE = K_PACK * J_STAGE   # 64
NB = T // STEPS_PER_STAGE            # 16 staging blocks

F32 = mybir.dt.float32


def _host_betas(t_steps, grid1, spline_w1, base_w1, grid2, spline_w2, base_w2):
    """KAN beta(t) in float64 (cast to f32) matching reference semantics."""
    x = t_steps.astype(np.float64)                       # [T,1]
    def rbf(x, grid, sw, bw):
        base = x @ bw.T.astype(np.float64)
        diff = x[:, :, None] - grid.astype(np.float64)[None, None, :]
        basis = np.exp(-(diff * diff) * 10.0).reshape(x.shape[0], -1)
        return base + basis @ sw.astype(np.float64)
    h = rbf(x, grid1, spline_w1, base_w1)                # [T,32]
    pre = rbf(h, grid2, spline_w2, base_w2)              # [T,1]
    betas = np.logaddexp(pre, 0.0)                       # softplus
    return betas.reshape(-1)                             # [T] f64


def _build_nc(c_imm: float, n_blocks: int = NB):
    nc = bacc.Bacc("TRN2", target_bir_lowering=False, debug=False,
                   num_devices=NCORES)
    t_total = n_blocks * STEPS_PER_STAGE

    i0_h = nc.dram_tensor("i0", [P, F], F32, kind="ExternalInput")
    s0_h = nc.dram_tensor("s0", [P, F], F32, kind="ExternalInput")
    db_h = nc.dram_tensor("db", [P, T], F32, kind="ExternalInput")
    ndb_h = nc.dram_tensor("ndb", [P, T], F32, kind="ExternalInput")
    out_h = nc.dram_tensor("out", [t_total, BL], F32, kind="ExternalOutput")
    ident_h = nc.inline_tensor(np.eye(P, dtype=np.float32), "ident")

    with tile.TileContext(nc) as tc:
        with (
            tc.tile_pool(name="const", bufs=1) as const,
            tc.tile_pool(name="state", bufs=3) as state,
            tc.tile_pool(name="pack", bufs=3) as packp,
            tc.tile_pool(name="tp", bufs=4, space="PSUM") as psump,
            tc.tile_pool(name="stage", bufs=2) as stagep,
            tc.tile_pool(name="acc", bufs=1) as accp,
        ):
            db_t = const.tile([P, T], F32, tag="db")
            nc.sync.dma_start(db_t[:], db_h.ap()[:])
            ndb_t = const.tile([P, T], F32, tag="ndb")
            nc.sync.dma_start(ndb_t[:], ndb_h.ap()[:])
            id_t = const.tile([P, P], F32, tag="id")
            nc.sync.dma_start(id_t[:], ident_h.ap()[:])
            i0_t = const.tile([P, F], F32, tag="i0")
            nc.sync.dma_start(i0_t[:], i0_h.ap()[:])

            s_cur = state.tile([P, F], F32, tag="S")
            nc.sync.dma_start(s_cur[:], s0_h.ap()[:])

            acc1 = accp.tile([P, 1], F32, tag="a1")
            acc2 = accp.tile([P, 1], F32, tag="a2")

            # out[t, b] viewed as [nb, k, f, j, p]: t = nb*64 + j*4 + k,
            # b = f*128 + p. SBUF staging is [part q=(k,f), free=(j,p)].
            ov = out_h.ap().rearrange(
                "(nb j k) (f p) -> nb k f j p", j=J_STAGE, k=K_PACK, p=P
            )

            i_cur = i0_t[:]
            for nb in range(n_blocks):
                st = stagep.tile([P, J_STAGE * P], F32, tag="st")
                for jj in range(J_STAGE):
                    pk = packp.tile([P, P], F32, tag="pk")
                    for k in range(K_PACK):
                        t = nb * STEPS_PER_STAGE + jj * K_PACK + k
                        i_new = pk[:, k * F:(k + 1) * F]
                        nc.vector.affine_mul_reduce(
                            out=i_new, accum_out=acc1[:],
                            in0=s_cur[:], in1=i_cur,
                            scale=db_t[:, t:t + 1], bias=c_imm,
                        )
                        s_new = state.tile([P, F], F32, tag="S")
                        nc.vector.affine_mul_reduce(
                            out=s_new[:], accum_out=acc2[:],
                            in0=i_cur, in1=s_cur[:],
                            scale=ndb_t[:, t:t + 1], bias=1.0,
                        )
                        i_cur = i_new
                        s_cur = s_new
                    tp = psump.tile([P, P], F32, tag="tp")
                    nc.tensor.transpose(tp[:], pk[:], id_t[:])
                    nc.scalar.copy(st[:, jj * P:(jj + 1) * P], tp[:])
                for k in range(K_PACK):
                    src = st[k * F:(k + 1) * F, :].rearrange(
                        "f (j p) -> f j p", p=P
                    )
                    nc.sync.dma_start(ov[nb, k], src)
    nc.compile()
    return nc


def kernel(t_steps, initial_I, grid1, spline_w1, base_w1, grid2, spline_w2,
           base_w2, gamma_param, _trace=False):
    t_steps = np.asarray(t_steps)
    initial_I = np.asarray(initial_I, dtype=np.float32)
    betas = _host_betas(np.asarray(t_steps), np.asarray(grid1),
                        np.asarray(spline_w1), np.asarray(base_w1),
                        np.asarray(grid2), np.asarray(spline_w2),
                        np.asarray(base_w2))
    dt = float(np.float32(t_steps[1, 0]) - np.float32(t_steps[0, 0]))
    gamma = float(np.logaddexp(np.asarray(gamma_param, np.float64)[0], 0.0))
    c_imm = float(np.float32(1.0 - dt * gamma))

    db = (betas * dt).astype(np.float32)                     # [T]
    db_b = np.ascontiguousarray(np.broadcast_to(db, (P, T)))
    ndb_b = np.ascontiguousarray(np.broadcast_to(-db, (P, T)))

    nc = _build_nc(c_imm)

    in_maps = []
    for c in range(NCORES):
        i0c = initial_I[c * BL:(c + 1) * BL].reshape(F, P).T  # [p,f]=b f*128+p
        s0c = (np.float32(1.0) - i0c).astype(np.float32)
        in_maps.append({
            "i0": np.ascontiguousarray(i0c),
            "s0": np.ascontiguousarray(s0c),
            "db": db_b,
            "ndb": ndb_b,
        })

    res = run_bass_kernel_spmd(nc, in_maps, core_ids=list(range(NCORES)),
                               trace=_trace)
    out = np.concatenate([res.results[c]["out"] for c in range(NCORES)],
                         axis=1)
    if _trace:
        kernel._last_result = res
    return out

